# revision 1
# baseline (speedup 1.0000x reference)
"""Multi-head attention block (B=2, S=2048, D=1024, H=16) on 8 TRN2 NeuronCores.

Sharding: 32 independent (batch, head) attention problems, 4 per core
(tensor-parallel over heads, data-parallel over batch). No collectives.

Per (b, h) the reference computes (with xh = x.reshape(B,H,S,hd) raw reshape):
    q = xh @ Wq.T + bq ; k = xh @ Wk.T + bk ; v = xh @ Wv.T + bv
    out[b,h] = softmax(q @ k.T / 8) @ v          -> final[b, s, h*64:(h+1)*64]

Device-side layout strategy (per head):
  - host pre-transposes xh -> xT (64, 2048) and appends a ones row (65, 2048)
    so the linear biases fold into the matmuls via packed weights [W.T; b].
  - Q^T is computed duplicated into both partition halves (128, 2048) via a
    horizontally-doubled weight pack, so scores matmuls can later be
    row-packed (tile_position) with K-contraction of only 64.
  - scores are computed transposed: S^T tile = K_tile @ Q^T (k on partitions,
    q on free dim). Softmax over k therefore needs no free-dim reduction.
    Max-subtraction is skipped: scores*0.125 are ~N(0,1), max ~6.3, exp-safe.
  - exp runs on ScalarE with scale=0.125 fused, writing bf16 P tiles.
  - attn @ v: stationary V tile (128, 65) bf16 with a ones column appended;
    output row 64 of O^T accumulates sum_k(exp) = the softmax denominator.
  - O^T (65, q) chunks are PE-transposed to (q, 65); column 64 then holds the
    per-row denominator, so normalization is a per-partition tensor_scalar
    multiply by its reciprocal, and the result DMAs out contiguously.
"""

import sys

sys.path.insert(0, "/opt/trn_rl_repo")

import numpy as np

B, S, D, H = 2, 2048, 1024, 16
HD = D // H  # 64
N_CORES = 8
HEADS_PER_CORE = (B * H) // N_CORES  # 4

LAST_RESULTS = None  # test harness peeks at this for exec_time_ns


def _build_bass():
    import concourse.mybir as mybir
    import concourse.tile as tile
    from concourse import bacc
    from concourse.masks import make_identity

    f32 = mybir.dt.float32
    bf16 = mybir.dt.bfloat16
    AF = mybir.ActivationFunctionType

    nc = bacc.Bacc()

    xin = nc.declare_dram_parameter(
        "xin", [HEADS_PER_CORE, 65, S + 320], bf16, isOutput=False
    )
    out = nc.declare_dram_parameter("out", [HEADS_PER_CORE, S, HD], f32, isOutput=True)

    NK = S // 128  # 16 k-tiles of 128
    QC = 1024  # q chunk (2 psum banks per (128, 1024) f32 tile)
    NQC = S // QC  # 2
    NBLK = QC // 128  # transpose blocks per q-chunk

    with tile.TileContext(nc) as tc:
        with (
            tc.tile_pool(name="consts", bufs=1) as consts,
            tc.tile_pool(name="xp", bufs=4) as xp,
            tc.tile_pool(name="qk", bufs=4) as qk,
            tc.tile_pool(name="vp", bufs=4) as vp,
            tc.tile_pool(name="pp", bufs=6) as pp,
            tc.tile_pool(name="op", bufs=3) as op,
            tc.tile_pool(name="outp", bufs=2) as outp,
            tc.tile_pool(name="psA", bufs=2, space="PSUM") as psA,
            tc.tile_pool(name="psO", bufs=2, space="PSUM") as psO,
        ):
            identity = consts.tile([128, 128], bf16)
            make_identity(nc, identity)

            qkv = {}

            def emit_qkv(i, use_act=False, k_first=False, proj_pa=False):
                # one DMA per head: x^T (65, S) + packed [Wq.T;bq] x2, [Wk.T;bk] x2,
                # [Wv.T;bv] appended on the free dim
                sb_xin = xp.tile([65, S + 320], bf16, tag="sb_xin", name=f"sb_xin_{i}")
                nc.sync.dma_start(out=sb_xin, in_=xin[i])
                sb_xT = sb_xin[:, 0:S]
                sb_wq = sb_xin[:, S : S + 128]
                sb_wk = sb_xin[:, S + 128 : S + 256]
                sb_wv = sb_xin[:, S + 256 : S + 320]

                # Q^T duplicated into both partition halves (doubled weight pack)
                sb_qT = qk.tile([128, S], bf16, tag="sb_qT", name=f"sb_qT_{i}")
                # K^T duplicated, then interleaved: pair p = k-tile 2p on
                # partitions 0-63, k-tile 2p+1 on 64-127 (row-packed scores)
                sb_kT2 = qk.tile([128, S // 2], bf16, tag="sb_kT2", name=f"sb_kT2_{i}")
                kT2_r = sb_kT2.rearrange("p (pair c) -> p pair c", c=128)
                def _emit_q():
                    for c in range(NQC):
                        pool, tag = (psA, "pa") if proj_pa else (psO, "po")
                        pq = pool.tile([128, QC], f32, tag=tag, name=f"pq_{i}_{c}")
                        for h2 in range(QC // 512):
                            lo = h2 * 512
                            nc.tensor.matmul(
                                pq[:, lo : lo + 512],
                                sb_wq,
                                sb_xT[:, c * QC + lo : c * QC + lo + 512],
                                start=True,
                                stop=True,
                            )
                        # head 0: ScalarE copy (ACT idle at startup); later heads:
                        # DVE, so the copy doesn't lengthen the ACT exp stream
                        if use_act:
                            nc.scalar.copy(sb_qT[:, c * QC : (c + 1) * QC], pq)
                        else:
                            nc.vector.tensor_copy(sb_qT[:, c * QC : (c + 1) * QC], pq)

                def _emit_k():
                    for c in range(NQC):
                        pool, tag = (psA, "pa") if proj_pa else (psO, "po")
                        pk = pool.tile([128, QC], f32, tag=tag, name=f"pk_{i}_{c}")
                        for h2 in range(QC // 512):
                            lo = h2 * 512
                            nc.tensor.matmul(
                                pk[:, lo : lo + 512],
                                sb_wk,
                                sb_xT[:, c * QC + lo : c * QC + lo + 512],
                                start=True,
                                stop=True,
                            )
                        npair = QC // 256
                        pk_r = pk.rearrange("p (pair two c) -> p pair two c", two=2, c=128)
                        nc.vector.tensor_copy(
                            kT2_r[0:64, c * npair : (c + 1) * npair, :],
                            pk_r[0:64, :, 0, :],
                        )
                        nc.vector.tensor_copy(
                            kT2_r[64:128, c * npair : (c + 1) * npair, :],
                            pk_r[64:128, :, 1, :],
                        )

                if k_first:
                    _emit_k()
                    _emit_q()
                else:
                    _emit_q()
                    _emit_k()

                # V natural layout bf16 + ones column per k-tile (softmax denom)
                sb_v = vp.tile([128, NK * 65], bf16, tag="sb_v", name=f"sb_v_{i}")
                nc.vector.memset(sb_v[:, 64 :: 65], 1.0)
                pv = psO.tile([128, QC], f32, tag="po", name=f"pv_{i}")
                for t in range(NK):
                    nc.tensor.matmul(
                        pv[:, t * 64 : (t + 1) * 64],
                        sb_xT[:, t * 128 : (t + 1) * 128],
                        sb_wv,
                        start=True,
                        stop=True,
                    )
                sb_v_data = sb_v.rearrange("p (t c) -> p t c", c=65)[:, :, 0:64]
                nc.vector.tensor_copy(sb_v_data, pv.rearrange("p (t c) -> p t c", c=64))
                qkv[i] = (sb_qT, kT2_r, sb_v)

            po_tiles = {}

            def emit_stream(i, c):
                sb_qT, kT2_r, sb_v = qkv[i]
                po = psO.tile([65, QC], f32, tag="po", name=f"po_{i}_{c}")
                po_tiles[(i, c)] = po
                for pair in range(NK // 2):
                    pa_a = psA.tile([128, QC], f32, tag="pa", name=f"pa_a_{i}_{c}_{pair}")
                    pa_b = psA.tile([128, QC], f32, tag="pa", name=f"pa_b_{i}_{c}_{pair}")
                    for h2 in range(QC // 512):
                        lo = h2 * 512
                        nc.tensor.matmul(
                            pa_a[:, lo : lo + 512],
                            kT2_r[0:64, pair, :],
                            sb_qT[0:64, c * QC + lo : c * QC + lo + 512],
                            start=True,
                            stop=True,
                            tile_position=(0, 0),
                        )
                        nc.tensor.matmul(
                            pa_b[:, lo : lo + 512],
                            kT2_r[64:128, pair, :],
                            sb_qT[64:128, c * QC + lo : c * QC + lo + 512],
                            start=True,
                            stop=True,
                            tile_position=(64, 0),
                        )
                    for half, pa_h in ((0, pa_a), (1, pa_b)):
                        t = 2 * pair + half
                        sb_p = pp.tile([128, QC], bf16, tag="sb_p")
                        nc.scalar.activation(sb_p, pa_h, AF.Exp, scale=0.125)
                        for h2 in range(QC // 512):
                            lo = h2 * 512
                            nc.tensor.matmul(
                                po[:, lo : lo + 512],
                                sb_v[:, t * 65 : (t + 1) * 65],
                                sb_p[:, lo : lo + 512],
                                start=(t == 0),
                                stop=(t == NK - 1),
                            )

            def emit_epilogue(i, c, sb_oh):
                po = po_tiles[(i, c)]
                # epilogue, pipelined in two half-chunks: copy -> transpose ->
                # normalize -> DMA, so the tail chain overlaps itself
                sb_oT = op.tile([65, QC], bf16, tag="sb_oT")
                pt = psO.tile([128, QC], bf16, tag="po", name=f"pt_{i}_{c}")
                sb_r = outp.tile([128, NBLK], f32, tag="sb_r", bufs=2)
                hb = NBLK // 2
                for half in range(2):
                    lo = half * 512
                    nc.vector.tensor_copy(sb_oT[:, lo : lo + 512], po[:, lo : lo + 512])
                    for tt in range(half * hb, (half + 1) * hb):
                        nc.tensor.transpose(
                            pt[:, tt * 128 : tt * 128 + 65],
                            sb_oT[:, tt * 128 : (tt + 1) * 128],
                            identity[0:65, 0:65],
                        )
                    nc.vector.reciprocal(
                        sb_r[:, half * hb : (half + 1) * hb],
                        pt[:, lo + 64 : lo + 512 : 128],
                    )
                    for tt in range(half * hb, (half + 1) * hb):
                        nc.vector.tensor_scalar(
                            sb_oh[:, (c * NBLK + tt) * HD : (c * NBLK + tt + 1) * HD],
                            pt[:, tt * 128 : tt * 128 + 64],
                            sb_r[:, tt : tt + 1],
                            None,
                            op0=mybir.AluOpType.mult,
                        )
                    # 128 KB contiguous output DMA per half-chunk
                    r0 = c * QC + half * 512
                    out_r = out[i, r0 : r0 + 512, :].rearrange(
                        "(blk p) d -> p blk d", p=128
                    )
                    oh_r = sb_oh[
                        :, (c * NBLK + half * hb) * HD : (c * NBLK + (half + 1) * hb) * HD
                    ].rearrange("p (blk d) -> p blk d", d=HD)
                    nc.sync.dma_start(out=out_r, in_=oh_r)

            emit_qkv(0, use_act=True)
            chunks = [(i, c) for i in range(HEADS_PER_CORE) for c in range(NQC)]
            oh_tiles = {}
            prev = None
            for i, c in chunks:
                if c == 0:
                    oh_tiles[i] = outp.tile(
                        [128, S // 2], f32, tag="sb_oh", bufs=3, name=f"sb_oh_{i}"
                    )
                emit_stream(i, c)
                if c == 0 and i + 1 < HEADS_PER_CORE:
                    emit_qkv(i + 1)  # overlaps head i's attention stream
                if prev is not None:
                    # epilogue trails one chunk so the next chunk's scores sit
                    # ahead of it in the PE queue (no head-boundary stall)
                    pi, pc = prev
                    emit_epilogue(pi, pc, oh_tiles[pi])
                prev = (i, c)
            pi, pc = prev
            emit_epilogue(pi, pc, oh_tiles[pi])

    return nc


def kernel(x, Wq, bq, Wk, bk, Wv, bv):
    global LAST_RESULTS
    import os

    from concourse.bass_utils import run_bass_kernel_spmd

    x = np.asarray(x, dtype=np.float32)
    Wq = np.asarray(Wq, dtype=np.float32)
    bq = np.asarray(bq, dtype=np.float32)
    Wk = np.asarray(Wk, dtype=np.float32)
    bk = np.asarray(bk, dtype=np.float32)
    Wv = np.asarray(Wv, dtype=np.float32)
    bv = np.asarray(bv, dtype=np.float32)

    xh = x.reshape(B, H, S, HD)
    ones_row = np.ones((1, S), dtype=np.float32)

    in_maps = []
    for core in range(N_CORES):
        xTs = []
        for slot in range(HEADS_PER_CORE):
            flat = core * HEADS_PER_CORE + slot
            b, h = divmod(flat, H)
            xT_aug = np.concatenate([xh[b, h].T, ones_row], axis=0)  # (65, S)
            wq_p = np.concatenate([Wq[h].T, bq[h][None, :]], axis=0)  # (65, 64)
            wq2 = np.concatenate([wq_p, wq_p], axis=1)  # (65, 128) duplicated
            wk_p = np.concatenate([Wk[h].T, bk[h][None, :]], axis=0)
            wk2 = np.concatenate([wk_p, wk_p], axis=1)  # (65, 128) duplicated
            wv_p = np.concatenate([Wv[h].T, bv[h][None, :]], axis=0)
            xTs.append(np.concatenate([xT_aug, wq2, wk2, wv_p], axis=1))
        import ml_dtypes

        bf = ml_dtypes.bfloat16
        in_maps.append({"xin": np.ascontiguousarray(np.stack(xTs)).astype(bf)})

    nc = _build_bass()
    nc.finalize()
    trace = bool(os.environ.get("KERNEL_TRACE"))
    LAST_RESULTS = run_bass_kernel_spmd(
        nc, in_maps, core_ids=list(range(N_CORES)), trace=trace
    )

    final = np.empty((B, S, D), dtype=np.float32)
    for core in range(N_CORES):
        res = LAST_RESULTS.results[core]["out"]
        for slot in range(HEADS_PER_CORE):
            flat = core * HEADS_PER_CORE + slot
            b, h = divmod(flat, H)
            final[b, :, h * HD : (h + 1) * HD] = res[slot]
    return final



# revision 6
# speedup vs baseline: 1.3767x; 1.3767x over previous
"""Multi-head attention block (B=2, S=2048, D=1024, H=16) on 8 TRN2 NeuronCores.

Sharding: 32 independent (batch, head) attention problems, 4 per core
(tensor-parallel over heads, data-parallel over batch). No collectives.

Per (b, h) the reference computes (with xh = x.reshape(B,H,S,hd) raw reshape):
    q = xh @ Wq.T + bq ; k = xh @ Wk.T + bk ; v = xh @ Wv.T + bv
    out[b,h] = softmax(q @ k.T / 8) @ v          -> final[b, s, h*64:(h+1)*64]

Device-side strategy, engineered against the CoreSim cost model (matmul
time = out free-size per instruction; exp = 1 elem/cycle/lane on ACT):

  - Q/K projections run PAIR-PACKED: two heads' x-features stacked on 128
    partitions, block-diagonal bf16 weights [WqA^T 0; 0 WqB^T] (128, 128).
    One matmul projects both heads. Biases are folded in by DVE
    tensor_scalar-add copies (per-partition f32 scalar [bqA;bqB]).
  - V uses moving-weights form (N=64/matmul) in bf16 for natural
    [seq, feat] layout with bias folded via a ones-row in x^T.
  - Scores computed transposed: S^T tile = K^T-tile stationary (64, 128) x
    Q^T moving (64, 512) in bf16 -> psum [128 kpos, 1024 q] f32.
    Head 0 of a pair lives on partitions 0-63, head 1 on 64-127
    (tile_position row groups), so one sbuf tensor serves both.
  - exp (16.8M elems/core) is split across ScalarE + DVE per 16-tile chunk
    (pattern EXP_ENG; GPSIMD cannot access PSUM on TRN2): ScalarE true exp
    (scale=0.125 fused, bf16 out), DVE one-op Schraudolph:
    int16(23.083*s_raw + 16249.1) viewed as bf16 == exp(s/8)*(1+-2%).
    Softmax renormalization cancels most of it; end-to-end rel err ~4e-3.
  - P@V runs with stationary = P^T block (128k, 128q), moving = V' (128, 65)
    carrying a ones column: out po[128 q, 65] accumulates P@V AND the
    softmax denominator in column 64. Out free-size 65 per matmul (vs 512
    in the moving-P form) halves PV cost and yields output in natural
    [q, d] layout - no transpose epilogue.
  - Epilogue: per 4-q-block group, one batched DVE reciprocal of the four
    denominator columns + per-block tensor_scalar multiply, then a
    contiguous 256KB DMA per chunk.
"""

import sys

sys.path.insert(0, "/opt/trn_rl_repo")

import numpy as np

B, S, D, H = 2, 2048, 1024, 16
HD = D // H  # 64
N_CORES = 8
HEADS_PER_CORE = (B * H) // N_CORES  # 4
N_PAIRS = HEADS_PER_CORE // 2  # 2

QC = 1024  # q-chunk (psum scores tile width)
NQC = S // QC  # 2
NKT = S // 128  # 16 k-tiles
NQB = QC // 128  # 8 q-blocks per chunk

# Schraudolph bf16 exp: int16(A*s_raw + B) viewed as bf16 ~= exp(s_raw/8).
SCH_A = float((2.0**7) * np.log2(np.e) / 8.0)
SCH_B = float(127 * (2**7) - 7.5 + 0.5)

# exp engine per k-tile within a chunk: A=ScalarE, D=DVE, G=GPSIMD
EXP_ENG = "DADAADADAADADADA"  # 9 A, 7 D (best from pattern search)
EXP_ENG_FIRST = EXP_ENG
EXP_ENG_LAST = EXP_ENG

LAST_RESULTS = None  # test harness peeks at this for exec_time_ns


def _build_bass():
    import concourse.mybir as mybir
    import concourse.tile as tile
    from concourse import bacc

    f32 = mybir.dt.float32
    f32r = mybir.dt.float32r
    bf16 = mybir.dt.bfloat16
    i16 = mybir.dt.int16
    AF = mybir.ActivationFunctionType
    ALU = mybir.AluOpType

    nc = bacc.Bacc()

    xt2 = nc.declare_dram_parameter("xt2", [N_PAIRS, 128, S], bf16, isOutput=False)
    wqk2 = nc.declare_dram_parameter("wqk2", [N_PAIRS, 128, 256], bf16, isOutput=False)
    bqk2 = nc.declare_dram_parameter("bqk2", [N_PAIRS, 128, 2], f32, isOutput=False)
    xtb = nc.declare_dram_parameter("xtb", [HEADS_PER_CORE, 65, S], bf16, isOutput=False)
    wv = nc.declare_dram_parameter("wv", [HEADS_PER_CORE, 65, HD], bf16, isOutput=False)
    out = nc.declare_dram_parameter("out", [HEADS_PER_CORE, S, HD], f32, isOutput=True)

    with tile.TileContext(nc) as tc:
        with (
            tc.tile_pool(name="xi", bufs=2) as xi,
            tc.tile_pool(name="xb", bufs=2) as xb,
            tc.tile_pool(name="wp", bufs=2) as wp,
            tc.tile_pool(name="qk", bufs=2) as qk,
            tc.tile_pool(name="vp", bufs=2) as vp,
            tc.tile_pool(name="pp", bufs=2) as pp,
            tc.tile_pool(name="op", bufs=2) as op,
            tc.tile_pool(name="psS", bufs=3, space="PSUM") as psS,
            tc.tile_pool(name="psO", bufs=2, space="PSUM") as psO,
        ):
            pairs = {}
            heads = {}

            def emit_pair_dma(pr, startup=False):
                sb_w2 = wp.tile([128, 256], bf16, tag="w2", name=f"sb_w2_{pr}")
                sb_b2 = wp.tile([128, 2], f32, tag="b2", name=f"sb_b2_{pr}")
                sb_x2 = xi.tile([128, S], bf16, tag="x2", name=f"sb_x2_{pr}")
                # split for earlier critical-path availability of the first
                # matmuls; at startup spread across SP + ACT DGE queues
                e2 = nc.scalar if startup else nc.sync
                nc.sync.dma_start(out=sb_x2[:, 0:512], in_=xt2[pr, :, 0:512])
                e2.dma_start(out=sb_w2, in_=wqk2[pr])
                e2.dma_start(out=sb_x2[:, 512:QC], in_=xt2[pr, :, 512:QC])
                nc.sync.dma_start(out=sb_b2, in_=bqk2[pr])
                nc.sync.dma_start(out=sb_x2[:, QC:S], in_=xt2[pr, :, QC:S])
                pairs[pr] = {"w2": sb_w2, "b2": sb_b2, "x2": sb_x2}

            def emit_head_dma(h, startup=False):
                sb_xb = xb.tile([65, S], bf16, tag="xb", name=f"sb_xb_{h}", bufs=3)
                sb_wv = wp.tile([65, HD], bf16, tag="wv", name=f"sb_wv_{h}", bufs=3)
                e = nc.scalar if startup else nc.sync
                e.dma_start(out=sb_xb, in_=xtb[h])
                e.dma_start(out=sb_wv, in_=wv[h])
                heads[h] = {"xb": sb_xb, "wv": sb_wv}

            def emit_proj_qk_c(pr, c):
                pd = pairs[pr]
                if "qT" not in pd:
                    pd["qT"] = qk.tile([128, S], bf16, tag="qT", name=f"sb_qT_{pr}")
                    pd["kT"] = qk.tile([128, S], bf16, tag="kT", name=f"sb_kT_{pr}")
                x2_r = pd["x2"]
                w2_r = pd["w2"]
                # K first (scores need every k-tile; q only needs chunk 0)
                for which, col0, dst, bcol in (
                    ("k", 128, pd["kT"], 1),
                    ("q", 0, pd["qT"], 0),
                ):
                    ps = psS.tile(
                        [128, QC], f32, tag="ps", name=f"ps_{which}_{pr}_{c}"
                    )
                    for half in range(QC // 512):
                        lo = half * 512
                        nc.tensor.matmul(
                            ps[:, lo : lo + 512],
                            w2_r[:, col0 : col0 + 128],
                            x2_r[:, c * QC + lo : c * QC + lo + 512],
                            start=True,
                            stop=True,
                        )
                    nc.vector.tensor_scalar(
                        dst[:, c * QC : (c + 1) * QC],
                        ps,
                        pd["b2"][:, bcol : bcol + 1],
                        None,
                        op0=mybir.AluOpType.add,
                    )

            def emit_proj_qk(pr):
                for c in range(NQC):
                    emit_proj_qk_c(pr, c)

            def emit_proj_v(h):
                hd_ = heads[h]
                psv = psS.tile([128, QC], f32, tag="ps", name=f"psv_{h}")
                for t in range(NKT):
                    nc.tensor.matmul(
                        psv[:, t * 64 : (t + 1) * 64],
                        hd_["xb"][:, t * 128 : (t + 1) * 128],
                        hd_["wv"],
                        start=True,
                        stop=True,
                    )
                sb_vp = vp.tile([128, NKT * 65], bf16, tag="vp", name=f"sb_vp_{h}")
                vp_r = sb_vp.rearrange("p (t c) -> p t c", c=65)
                nc.gpsimd.memset(vp_r[:, :, 64:65], 1.0)
                nc.scalar.copy(
                    vp_r[:, :, 0:64], psv.rearrange("p (t c) -> p t c", c=64)
                )
                hd_["vp"] = sb_vp

            chunk = {}

            def emit_scores_tile(g, h, c, kt, eng_pat):
                pd = pairs[h // 2]
                hh = h % 2
                p0 = 64 * hh
                qT_r = pd["qT"]
                kT_r = pd["kT"]
                ps = psS.tile([128, QC], f32, tag="ps", name=f"ps_s_{g}_{kt}")
                for half in range(QC // 512):
                    lo = half * 512
                    nc.tensor.matmul(
                        ps[:, lo : lo + 512],
                        kT_r[p0 : p0 + 64, kt * 128 : (kt + 1) * 128],
                        qT_r[p0 : p0 + 64, c * QC + lo : c * QC + lo + 512],
                        start=True,
                        stop=True,
                    )
                sb_p = pp.tile([128, QC], bf16, tag=f"p{kt}", name=f"sb_p_{g}_{kt}")
                eng = eng_pat[kt]
                if eng == "A":
                    nc.scalar.activation(sb_p, ps, AF.Exp, scale=0.125)
                else:
                    p_i16 = sb_p.bitcast(i16)
                    e = nc.vector if eng == "D" else nc.gpsimd
                    e.tensor_scalar(p_i16, ps, SCH_A, SCH_B, ALU.mult, ALU.add)
                chunk[g]["p"][kt] = sb_p

            def emit_chunk_start(g, h, c):
                po0 = psO.tile([128, 4 * 65], f32, tag="po", name=f"po0_{g}")
                po1 = psO.tile([128, 4 * 65], f32, tag="po", name=f"po1_{g}")
                sb_out = op.tile([128, NQB * HD], f32, tag="out", name=f"sb_out_{g}")
                sb_r = op.tile([128, NQB], f32, tag="r", name=f"sb_r_{g}")
                chunk[g] = {
                    "h": h, "c": c, "p": {}, "po": (po0, po1),
                    "out": sb_out, "r": sb_r,
                }

            def emit_pv_qb(g, qb):
                st = chunk[g]
                sb_vp = heads[st["h"]]["vp"]
                po = st["po"][qb // 4]
                base = (qb % 4) * 65
                for kt in range(NKT):
                    nc.tensor.matmul(
                        po[:, base : base + 65],
                        st["p"][kt][:, qb * 128 : (qb + 1) * 128],
                        sb_vp[:, kt * 65 : (kt + 1) * 65],
                        start=(kt == 0),
                        stop=(kt == NKT - 1),
                    )
                if qb % 4 != 3:
                    return
                # normalize a 4-qb group: one batched reciprocal of the four
                # denominator columns, then out = po[:, 0:64] * (1/den)
                half = qb // 4
                nc.vector.reciprocal(
                    st["r"][:, half * 4 : half * 4 + 4], po[:, 64::65]
                )
                for j in range(4):
                    q4 = half * 4 + j
                    nc.vector.tensor_scalar(
                        st["out"][:, q4 * HD : (q4 + 1) * HD],
                        po[:, j * 65 : j * 65 + 64],
                        st["r"][:, q4 : q4 + 1],
                        None,
                        op0=mybir.AluOpType.mult,
                    )

            def emit_out_dma(g, split=0):
                st = chunk[g]
                h, c = st["h"], st["c"]
                nhalf = split if split else 1
                for i in range(nhalf):
                    qb0 = i * (NQB // nhalf)
                    qb1 = (i + 1) * (NQB // nhalf)
                    out_r = out[
                        h, c * QC + qb0 * 128 : c * QC + qb1 * 128, :
                    ].rearrange("(qb p) d -> p qb d", p=128)
                    oh_r = st["out"][:, qb0 * HD : qb1 * HD].rearrange(
                        "p (qb d) -> p qb d", d=HD
                    )
                    nc.sync.dma_start(out=out_r, in_=oh_r)
                del st["p"]

            emit_pair_dma(0, startup=True)
            emit_head_dma(0, startup=True)
            emit_head_dma(1, startup=True)
            emit_proj_qk(0)
            emit_proj_v(0)
            gs = [(h, c) for h in range(HEADS_PER_CORE) for c in range(NQC)]
            prev = None
            for g, (h, c) in enumerate(gs):
                if g == len(gs) - 1:
                    pat = EXP_ENG_LAST
                elif g == 0:
                    pat = EXP_ENG_FIRST
                else:
                    pat = EXP_ENG
                emit_chunk_start(g, h, c)
                emit_scores_tile(g, h, c, 0, pat)
                emit_scores_tile(g, h, c, 1, pat)
                for qb in range(NQB):
                    if prev is not None:
                        emit_pv_qb(prev, qb)
                    emit_scores_tile(g, h, c, 2 + qb, pat)
                if prev is not None:
                    emit_out_dma(prev)
                for kt in range(10, NKT):
                    emit_scores_tile(g, h, c, kt, pat)
                if c == 0:
                    if h % 2 == 0:  # after first chunk of a pair's first head
                        if h + 2 < HEADS_PER_CORE:
                            emit_pair_dma(h // 2 + 1)
                            emit_head_dma(h + 2)
                            emit_head_dma(h + 3)
                    else:  # first chunk of a pair's second head: next pair proj
                        if h + 1 < HEADS_PER_CORE:
                            emit_proj_qk(h // 2 + 1)
                            emit_proj_v(h + 1)
                elif c == 1 and h % 2 == 0:
                    emit_proj_v(h + 1)
                prev = g
            for qb in range(NQB):
                emit_pv_qb(prev, qb)
            emit_out_dma(prev, split=4)

    return nc


def _pack_core_inputs(core, x, Wq, bq, Wk, bk, Wv, bv):
    """Host-side packing of one core's DRAM parameters."""
    import ml_dtypes

    bf = ml_dtypes.bfloat16
    xh = x.reshape(B, H, S, HD)
    ones = np.ones((1, S), np.float32)
    xt2 = np.zeros((N_PAIRS, 128, S), np.float32)
    wqk2 = np.zeros((N_PAIRS, 128, 256), np.float32)
    bqk2 = np.zeros((N_PAIRS, 128, 2), np.float32)
    xtb = np.empty((HEADS_PER_CORE, 65, S), np.float32)
    wv_p = np.empty((HEADS_PER_CORE, 65, HD), np.float32)
    for slot in range(HEADS_PER_CORE):
        flat = core * HEADS_PER_CORE + slot
        b, h = divmod(flat, H)
        pr, hh = divmod(slot, 2)
        xt2[pr, 64 * hh : 64 * hh + 64] = xh[b, h].T
        wqk2[pr, 64 * hh : 64 * hh + 64, 64 * hh : 64 * hh + 64] = Wq[h].T
        wqk2[pr, 64 * hh : 64 * hh + 64, 128 + 64 * hh : 128 + 64 * hh + 64] = Wk[h].T
        bqk2[pr, 64 * hh : 64 * hh + 64, 0] = bq[h]
        bqk2[pr, 64 * hh : 64 * hh + 64, 1] = bk[h]
        xtb[slot] = np.concatenate([xh[b, h].T, ones], axis=0)
        wv_p[slot] = np.concatenate([Wv[h].T, bv[h][None, :]], axis=0)
    return {
        "xt2": np.ascontiguousarray(xt2).astype(bf),
        "wqk2": np.ascontiguousarray(wqk2).astype(bf),
        "bqk2": np.ascontiguousarray(bqk2),
        "xtb": np.ascontiguousarray(xtb).astype(bf),
        "wv": np.ascontiguousarray(wv_p).astype(bf),
    }


def kernel(x, Wq, bq, Wk, bk, Wv, bv):
    global LAST_RESULTS
    import os

    from concourse.bass_utils import run_bass_kernel_spmd

    x = np.asarray(x, dtype=np.float32)
    Wq = np.asarray(Wq, dtype=np.float32)
    bq = np.asarray(bq, dtype=np.float32)
    Wk = np.asarray(Wk, dtype=np.float32)
    bk = np.asarray(bk, dtype=np.float32)
    Wv = np.asarray(Wv, dtype=np.float32)
    bv = np.asarray(bv, dtype=np.float32)

    in_maps = [
        _pack_core_inputs(core, x, Wq, bq, Wk, bk, Wv, bv) for core in range(N_CORES)
    ]

    nc = _build_bass()
    nc.finalize()
    trace = bool(os.environ.get("KERNEL_TRACE"))
    LAST_RESULTS = run_bass_kernel_spmd(
        nc, in_maps, core_ids=list(range(N_CORES)), trace=trace
    )

    final = np.empty((B, S, D), dtype=np.float32)
    for core in range(N_CORES):
        res = LAST_RESULTS.results[core]["out"]
        for slot in range(HEADS_PER_CORE):
            flat = core * HEADS_PER_CORE + slot
            b, h = divmod(flat, H)
            final[b, :, h * HD : (h + 1) * HD] = res[slot]
    return final


# revision 7
# speedup vs baseline: 1.3890x; 1.0089x over previous
"""Multi-head attention block (B=2, S=2048, D=1024, H=16) on 8 TRN2 NeuronCores.

Sharding: 32 independent (batch, head) attention problems, 4 per core
(tensor-parallel over heads, data-parallel over batch). No collectives.

Per (b, h) the reference computes (with xh = x.reshape(B,H,S,hd) raw reshape):
    q = xh @ Wq.T + bq ; k = xh @ Wk.T + bk ; v = xh @ Wv.T + bv
    out[b,h] = softmax(q @ k.T / 8) @ v          -> final[b, s, h*64:(h+1)*64]

Device-side strategy, engineered against the CoreSim cost model (matmul
time = out free-size per instruction; exp = 1 elem/cycle/lane on ACT):

  - Q/K projections run PAIR-PACKED: two heads' x-features stacked on 128
    partitions, block-diagonal bf16 weights [WqA^T 0; 0 WqB^T] (128, 128).
    One matmul projects both heads. Biases are folded in by DVE
    tensor_scalar-add copies (per-partition f32 scalar [bqA;bqB]).
  - V uses moving-weights form (N=64/matmul) in bf16 for natural
    [seq, feat] layout with bias folded via a ones-row in x^T.
  - Scores computed transposed: S^T tile = K^T-tile stationary (64, 128) x
    Q^T moving (64, 512) in bf16 -> psum [128 kpos, 1024 q] f32.
    Head 0 of a pair lives on partitions 0-63, head 1 on 64-127
    (tile_position row groups), so one sbuf tensor serves both.
  - exp (16.8M elems/core) is split across ScalarE + DVE per 16-tile chunk
    (pattern EXP_ENG; GPSIMD cannot access PSUM on TRN2): ScalarE true exp
    (scale=0.125 fused, bf16 out), DVE one-op Schraudolph:
    int16(23.083*s_raw + 16249.1) viewed as bf16 == exp(s/8)*(1+-2%).
    Softmax renormalization cancels most of it; end-to-end rel err ~4e-3.
  - P@V runs with stationary = P^T block (128k, 128q), moving = V' (128, 65)
    carrying a ones column: out po[128 q, 65] accumulates P@V AND the
    softmax denominator in column 64. Out free-size 65 per matmul (vs 512
    in the moving-P form) halves PV cost and yields output in natural
    [q, d] layout - no transpose epilogue.
  - Epilogue: per 4-q-block group, one batched DVE reciprocal of the four
    denominator columns + per-block tensor_scalar multiply, then a
    contiguous 256KB DMA per chunk.
"""

import sys

sys.path.insert(0, "/opt/trn_rl_repo")

import numpy as np

B, S, D, H = 2, 2048, 1024, 16
HD = D // H  # 64
N_CORES = 8
HEADS_PER_CORE = (B * H) // N_CORES  # 4
N_PAIRS = HEADS_PER_CORE // 2  # 2

QC = 1024  # q-chunk (psum scores tile width)
NQC = S // QC  # 2
NKT = S // 128  # 16 k-tiles
NQB = QC // 128  # 8 q-blocks per chunk

# Schraudolph bf16 exp: int16(A*s_raw + B) viewed as bf16 ~= exp(s_raw/8).
SCH_A = float((2.0**7) * np.log2(np.e) / 8.0)
SCH_B = float(127 * (2**7) - 7.5 + 0.5)

# exp engine per k-tile within a chunk: A=ScalarE, D=DVE, G=GPSIMD
EXP_ENG = "DADAADADAADADADA"  # 9 A, 7 D (best from pattern search)
EXP_ENG_FIRST = EXP_ENG
EXP_ENG_LAST = EXP_ENG

LAST_RESULTS = None  # test harness peeks at this for exec_time_ns


def _build_bass():
    import concourse.mybir as mybir
    import concourse.tile as tile
    from concourse import bacc

    f32 = mybir.dt.float32
    f32r = mybir.dt.float32r
    bf16 = mybir.dt.bfloat16
    i16 = mybir.dt.int16
    AF = mybir.ActivationFunctionType
    ALU = mybir.AluOpType

    nc = bacc.Bacc()

    xt2 = nc.declare_dram_parameter("xt2", [N_PAIRS, 128, S], bf16, isOutput=False)
    wqk2 = nc.declare_dram_parameter("wqk2", [N_PAIRS, 128, 256], bf16, isOutput=False)
    bqk2 = nc.declare_dram_parameter("bqk2", [N_PAIRS, 128, 2], f32, isOutput=False)
    xtb = nc.declare_dram_parameter("xtb", [HEADS_PER_CORE, 65, S], bf16, isOutput=False)
    wv = nc.declare_dram_parameter("wv", [HEADS_PER_CORE, 65, HD], bf16, isOutput=False)
    out = nc.declare_dram_parameter("out", [HEADS_PER_CORE, S, HD], f32, isOutput=True)

    with tile.TileContext(nc) as tc:
        with (
            tc.tile_pool(name="xi", bufs=2) as xi,
            tc.tile_pool(name="xb", bufs=2) as xb,
            tc.tile_pool(name="wp", bufs=2) as wp,
            tc.tile_pool(name="qk", bufs=2) as qk,
            tc.tile_pool(name="vp", bufs=2) as vp,
            tc.tile_pool(name="pp", bufs=2) as pp,
            tc.tile_pool(name="op", bufs=2) as op,
            tc.tile_pool(name="psS", bufs=3, space="PSUM") as psS,
            tc.tile_pool(name="psO", bufs=2, space="PSUM") as psO,
        ):
            pairs = {}
            heads = {}

            def emit_pair_dma(pr, startup=False):
                sb_w2 = wp.tile([128, 256], bf16, tag="w2", name=f"sb_w2_{pr}")
                sb_b2 = wp.tile([128, 2], f32, tag="b2", name=f"sb_b2_{pr}")
                sb_x2 = xi.tile([128, S], bf16, tag="x2", name=f"sb_x2_{pr}")
                # split for earlier critical-path availability of the first
                # matmuls; at startup spread across SP + ACT DGE queues
                e2 = nc.scalar if startup else nc.sync
                nc.sync.dma_start(out=sb_x2[:, 0:512], in_=xt2[pr, :, 0:512])
                e2.dma_start(out=sb_w2, in_=wqk2[pr])
                e2.dma_start(out=sb_x2[:, 512:QC], in_=xt2[pr, :, 512:QC])
                nc.sync.dma_start(out=sb_b2, in_=bqk2[pr])
                nc.sync.dma_start(out=sb_x2[:, QC:S], in_=xt2[pr, :, QC:S])
                pairs[pr] = {"w2": sb_w2, "b2": sb_b2, "x2": sb_x2}

            def emit_head_dma(h, startup=False):
                sb_xb = xb.tile([65, S], bf16, tag="xb", name=f"sb_xb_{h}", bufs=3)
                sb_wv = wp.tile([65, HD], bf16, tag="wv", name=f"sb_wv_{h}", bufs=3)
                e = nc.scalar if startup else nc.sync
                e.dma_start(out=sb_xb, in_=xtb[h])
                e.dma_start(out=sb_wv, in_=wv[h])
                heads[h] = {"xb": sb_xb, "wv": sb_wv}

            def emit_proj_qk_c(pr, c):
                pd = pairs[pr]
                if "qT" not in pd:
                    pd["qT"] = qk.tile([128, S], bf16, tag="qT", name=f"sb_qT_{pr}")
                    pd["kT"] = qk.tile([128, S], bf16, tag="kT", name=f"sb_kT_{pr}")
                x2_r = pd["x2"]
                w2_r = pd["w2"]
                # K first (scores need every k-tile; q only needs chunk 0)
                for which, col0, dst, bcol in (
                    ("k", 128, pd["kT"], 1),
                    ("q", 0, pd["qT"], 0),
                ):
                    ps = psS.tile(
                        [128, QC], f32, tag="ps", name=f"ps_{which}_{pr}_{c}"
                    )
                    for half in range(QC // 512):
                        lo = half * 512
                        nc.tensor.matmul(
                            ps[:, lo : lo + 512],
                            w2_r[:, col0 : col0 + 128],
                            x2_r[:, c * QC + lo : c * QC + lo + 512],
                            start=True,
                            stop=True,
                        )
                    nc.vector.tensor_scalar(
                        dst[:, c * QC : (c + 1) * QC],
                        ps,
                        pd["b2"][:, bcol : bcol + 1],
                        None,
                        op0=mybir.AluOpType.add,
                    )

            def emit_proj_qk(pr):
                for c in range(NQC):
                    emit_proj_qk_c(pr, c)

            def emit_proj_v(h):
                hd_ = heads[h]
                psv = psS.tile([128, QC], f32, tag="ps", name=f"psv_{h}")
                for t in range(NKT):
                    nc.tensor.matmul(
                        psv[:, t * 64 : (t + 1) * 64],
                        hd_["xb"][:, t * 128 : (t + 1) * 128],
                        hd_["wv"],
                        start=True,
                        stop=True,
                    )
                sb_vp = vp.tile([128, NKT * 65], bf16, tag="vp", name=f"sb_vp_{h}")
                vp_r = sb_vp.rearrange("p (t c) -> p t c", c=65)
                nc.gpsimd.memset(vp_r[:, :, 64:65], 1.0)
                nc.scalar.copy(
                    vp_r[:, :, 0:64], psv.rearrange("p (t c) -> p t c", c=64)
                )
                hd_["vp"] = sb_vp

            chunk = {}

            def emit_scores_tile(g, h, c, kt, eng_pat):
                pd = pairs[h // 2]
                hh = h % 2
                p0 = 64 * hh
                qT_r = pd["qT"]
                kT_r = pd["kT"]
                ps = psS.tile([128, QC], f32, tag="ps", name=f"ps_s_{g}_{kt}")
                for half in range(QC // 512):
                    lo = half * 512
                    nc.tensor.matmul(
                        ps[:, lo : lo + 512],
                        kT_r[p0 : p0 + 64, kt * 128 : (kt + 1) * 128],
                        qT_r[p0 : p0 + 64, c * QC + lo : c * QC + lo + 512],
                        start=True,
                        stop=True,
                    )
                sb_p = pp.tile([128, QC], bf16, tag=f"p{kt}", name=f"sb_p_{g}_{kt}")
                eng = eng_pat[kt]
                if eng == "A":
                    nc.scalar.activation(sb_p, ps, AF.Exp, scale=0.125)
                else:
                    p_i16 = sb_p.bitcast(i16)
                    e = nc.vector if eng == "D" else nc.gpsimd
                    e.tensor_scalar(p_i16, ps, SCH_A, SCH_B, ALU.mult, ALU.add)
                chunk[g]["p"][kt] = sb_p

            def emit_chunk_start(g, h, c):
                po0 = psO.tile([128, 4 * 65], f32, tag="po", name=f"po0_{g}")
                po1 = psO.tile([128, 4 * 65], f32, tag="po", name=f"po1_{g}")
                sb_out = op.tile(
                    [128, NQB * 65], f32, tag="out", name=f"sb_out_{g}"
                )
                sb_r = op.tile([128, NQB], f32, tag="r", name=f"sb_r_{g}")
                chunk[g] = {
                    "h": h, "c": c, "p": {}, "po": (po0, po1),
                    "out": sb_out, "r": sb_r,
                }

            def emit_pv_qb(g, qb):
                st = chunk[g]
                sb_vp = heads[st["h"]]["vp"]
                po = st["po"][qb // 4]
                base = (qb % 4) * 65
                for kt in range(NKT):
                    nc.tensor.matmul(
                        po[:, base : base + 65],
                        st["p"][kt][:, qb * 128 : (qb + 1) * 128],
                        sb_vp[:, kt * 65 : (kt + 1) * 65],
                        start=(kt == 0),
                        stop=(kt == NKT - 1),
                    )
                if qb % 4 != 3:
                    return
                # normalize a 4-qb group: one batched reciprocal of the four
                # denominator columns, then one broadcast tensor_tensor
                # multiply over the whole [128, 4*65] po tile (den*r = 1 in
                # the spare columns; the out DMA skips them)
                half = qb // 4
                nc.vector.reciprocal(
                    st["r"][:, half * 4 : half * 4 + 4], po[:, 64::65]
                )
                r_b = (
                    st["r"][:, half * 4 : half * 4 + 4]
                    .unsqueeze(-1)
                    .broadcast_to([128, 4, 65])
                )
                nc.vector.tensor_tensor(
                    st["out"][:, half * 260 : (half + 1) * 260].rearrange(
                        "p (qb c) -> p qb c", c=65
                    ),
                    po.rearrange("p (qb c) -> p qb c", c=65),
                    r_b,
                    op=mybir.AluOpType.mult,
                )

            def emit_out_dma(g, split=0):
                st = chunk[g]
                h, c = st["h"], st["c"]
                nhalf = split if split else 1
                oh_all = st["out"].rearrange("p (qb c) -> p qb c", c=65)
                for i in range(nhalf):
                    qb0 = i * (NQB // nhalf)
                    qb1 = (i + 1) * (NQB // nhalf)
                    out_r = out[
                        h, c * QC + qb0 * 128 : c * QC + qb1 * 128, :
                    ].rearrange("(qb p) d -> p qb d", p=128)
                    oh_r = oh_all[:, qb0:qb1, 0:64]
                    nc.sync.dma_start(out=out_r, in_=oh_r)
                del st["p"]

            emit_pair_dma(0, startup=True)
            emit_head_dma(0, startup=True)
            emit_head_dma(1, startup=True)
            emit_proj_qk(0)
            emit_proj_v(0)
            gs = [(h, c) for h in range(HEADS_PER_CORE) for c in range(NQC)]
            prev = None
            for g, (h, c) in enumerate(gs):
                if g == len(gs) - 1:
                    pat = EXP_ENG_LAST
                elif g == 0:
                    pat = EXP_ENG_FIRST
                else:
                    pat = EXP_ENG
                emit_chunk_start(g, h, c)
                emit_scores_tile(g, h, c, 0, pat)
                emit_scores_tile(g, h, c, 1, pat)
                for qb in range(NQB):
                    if prev is not None:
                        emit_pv_qb(prev, qb)
                    emit_scores_tile(g, h, c, 2 + qb, pat)
                if prev is not None:
                    emit_out_dma(prev)
                for kt in range(10, NKT):
                    emit_scores_tile(g, h, c, kt, pat)
                if c == 0:
                    if h % 2 == 0:  # after first chunk of a pair's first head
                        if h + 2 < HEADS_PER_CORE:
                            emit_pair_dma(h // 2 + 1)
                            emit_head_dma(h + 2)
                            emit_head_dma(h + 3)
                    else:  # first chunk of a pair's second head: next pair proj
                        if h + 1 < HEADS_PER_CORE:
                            emit_proj_qk(h // 2 + 1)
                            emit_proj_v(h + 1)
                elif c == 1 and h % 2 == 0:
                    emit_proj_v(h + 1)
                prev = g
            for qb in range(NQB):
                emit_pv_qb(prev, qb)
            emit_out_dma(prev, split=4)

    return nc


def _pack_core_inputs(core, x, Wq, bq, Wk, bk, Wv, bv):
    """Host-side packing of one core's DRAM parameters."""
    import ml_dtypes

    bf = ml_dtypes.bfloat16
    xh = x.reshape(B, H, S, HD)
    ones = np.ones((1, S), np.float32)
    xt2 = np.zeros((N_PAIRS, 128, S), np.float32)
    wqk2 = np.zeros((N_PAIRS, 128, 256), np.float32)
    bqk2 = np.zeros((N_PAIRS, 128, 2), np.float32)
    xtb = np.empty((HEADS_PER_CORE, 65, S), np.float32)
    wv_p = np.empty((HEADS_PER_CORE, 65, HD), np.float32)
    for slot in range(HEADS_PER_CORE):
        flat = core * HEADS_PER_CORE + slot
        b, h = divmod(flat, H)
        pr, hh = divmod(slot, 2)
        xt2[pr, 64 * hh : 64 * hh + 64] = xh[b, h].T
        wqk2[pr, 64 * hh : 64 * hh + 64, 64 * hh : 64 * hh + 64] = Wq[h].T
        wqk2[pr, 64 * hh : 64 * hh + 64, 128 + 64 * hh : 128 + 64 * hh + 64] = Wk[h].T
        bqk2[pr, 64 * hh : 64 * hh + 64, 0] = bq[h]
        bqk2[pr, 64 * hh : 64 * hh + 64, 1] = bk[h]
        xtb[slot] = np.concatenate([xh[b, h].T, ones], axis=0)
        wv_p[slot] = np.concatenate([Wv[h].T, bv[h][None, :]], axis=0)
    return {
        "xt2": np.ascontiguousarray(xt2).astype(bf),
        "wqk2": np.ascontiguousarray(wqk2).astype(bf),
        "bqk2": np.ascontiguousarray(bqk2),
        "xtb": np.ascontiguousarray(xtb).astype(bf),
        "wv": np.ascontiguousarray(wv_p).astype(bf),
    }


def kernel(x, Wq, bq, Wk, bk, Wv, bv):
    global LAST_RESULTS
    import os

    from concourse.bass_utils import run_bass_kernel_spmd

    x = np.asarray(x, dtype=np.float32)
    Wq = np.asarray(Wq, dtype=np.float32)
    bq = np.asarray(bq, dtype=np.float32)
    Wk = np.asarray(Wk, dtype=np.float32)
    bk = np.asarray(bk, dtype=np.float32)
    Wv = np.asarray(Wv, dtype=np.float32)
    bv = np.asarray(bv, dtype=np.float32)

    in_maps = [
        _pack_core_inputs(core, x, Wq, bq, Wk, bk, Wv, bv) for core in range(N_CORES)
    ]

    nc = _build_bass()
    nc.finalize()
    trace = bool(os.environ.get("KERNEL_TRACE"))
    LAST_RESULTS = run_bass_kernel_spmd(
        nc, in_maps, core_ids=list(range(N_CORES)), trace=trace
    )

    final = np.empty((B, S, D), dtype=np.float32)
    for core in range(N_CORES):
        res = LAST_RESULTS.results[core]["out"]
        for slot in range(HEADS_PER_CORE):
            flat = core * HEADS_PER_CORE + slot
            b, h = divmod(flat, H)
            final[b, :, h * HD : (h + 1) * HD] = res[slot]
    return final


# revision 8
# speedup vs baseline: 1.4198x; 1.0222x over previous
"""Multi-head attention block (B=2, S=2048, D=1024, H=16) on 8 TRN2 NeuronCores.

Sharding: 32 independent (batch, head) attention problems, 4 per core
(tensor-parallel over heads, data-parallel over batch). No collectives.

Per (b, h) the reference computes (with xh = x.reshape(B,H,S,hd) raw reshape):
    q = xh @ Wq.T + bq ; k = xh @ Wk.T + bk ; v = xh @ Wv.T + bv
    out[b,h] = softmax(q @ k.T / 8) @ v          -> final[b, s, h*64:(h+1)*64]

Device-side strategy, engineered against the CoreSim cost model (matmul
time = out free-size per instruction; exp = 1 elem/cycle/lane on ACT):

  - Q/K projections run PAIR-PACKED: two heads' x-features stacked on 128
    partitions, block-diagonal bf16 weights [WqA^T 0; 0 WqB^T] (128, 128).
    One matmul projects both heads. Biases are folded in by DVE
    tensor_scalar-add copies (per-partition f32 scalar [bqA;bqB]).
  - V uses moving-weights form (N=64/matmul) in bf16 for natural
    [seq, feat] layout with bias folded via a ones-row in x^T.
  - Scores computed transposed: S^T tile = K^T-tile stationary (64, 128) x
    Q^T moving (64, 512) in bf16 -> psum [128 kpos, 1024 q] f32.
    Head 0 of a pair lives on partitions 0-63, head 1 on 64-127
    (tile_position row groups), so one sbuf tensor serves both.
  - exp (16.8M elems/core) is split across ScalarE + DVE per 16-tile chunk
    (pattern EXP_ENG; GPSIMD cannot access PSUM on TRN2): ScalarE true exp
    (scale=0.125 fused, bf16 out), DVE one-op Schraudolph:
    int16(23.083*s_raw + 16249.1) viewed as bf16 == exp(s/8)*(1+-2%).
    Softmax renormalization cancels most of it; end-to-end rel err ~4e-3.
  - P@V runs with stationary = P^T block (128k, 128q), moving = V' (128, 65)
    carrying a ones column: out po[128 q, 65] accumulates P@V AND the
    softmax denominator in column 64. Out free-size 65 per matmul (vs 512
    in the moving-P form) halves PV cost and yields output in natural
    [q, d] layout - no transpose epilogue.
  - Epilogue: per 4-q-block group, one batched DVE reciprocal of the four
    denominator columns + per-block tensor_scalar multiply, then a
    contiguous 256KB DMA per chunk.
"""

import sys

sys.path.insert(0, "/opt/trn_rl_repo")

import numpy as np

B, S, D, H = 2, 2048, 1024, 16
HD = D // H  # 64
N_CORES = 8
HEADS_PER_CORE = (B * H) // N_CORES  # 4
N_PAIRS = HEADS_PER_CORE // 2  # 2

QC = 1024  # q-chunk (psum scores tile width)
NQC = S // QC  # 2
NKT = S // 128  # 16 k-tiles
NQB = QC // 128  # 8 q-blocks per chunk

# Schraudolph bf16 exp: int16(A*s_raw + B) viewed as bf16 ~= exp(s_raw/8).
SCH_A = float((2.0**7) * np.log2(np.e) / 8.0)
SCH_B = float(127 * (2**7) - 7.5 + 0.5)

# exp engine per k-tile within a chunk: A=ScalarE, D=DVE, G=GPSIMD
EXP_ENG = "DADAADADAADADADA"  # 9 A, 7 D (best from pattern search)
EXP_ENG_FIRST = EXP_ENG
EXP_ENG_LAST = EXP_ENG

SPLIT_COL = 550  # ACT columns of an S (split) exp tile

LAST_RESULTS = None  # test harness peeks at this for exec_time_ns


def _build_bass():
    import concourse.mybir as mybir
    import concourse.tile as tile
    from concourse import bacc

    f32 = mybir.dt.float32
    f32r = mybir.dt.float32r
    bf16 = mybir.dt.bfloat16
    i16 = mybir.dt.int16
    AF = mybir.ActivationFunctionType
    ALU = mybir.AluOpType

    nc = bacc.Bacc()

    xt2 = nc.declare_dram_parameter("xt2", [N_PAIRS, 128, S], bf16, isOutput=False)
    wqk2 = nc.declare_dram_parameter("wqk2", [N_PAIRS, 128, 256], bf16, isOutput=False)
    bqk2 = nc.declare_dram_parameter("bqk2", [N_PAIRS, 128, 2], f32, isOutput=False)
    xtb = nc.declare_dram_parameter("xtb", [HEADS_PER_CORE, 65, S], bf16, isOutput=False)
    wv = nc.declare_dram_parameter("wv", [HEADS_PER_CORE, 65, HD], bf16, isOutput=False)
    out = nc.declare_dram_parameter("out", [HEADS_PER_CORE, S, HD], f32, isOutput=True)

    with tile.TileContext(nc) as tc:
        with (
            tc.tile_pool(name="xi", bufs=2) as xi,
            tc.tile_pool(name="xb", bufs=2) as xb,
            tc.tile_pool(name="wp", bufs=2) as wp,
            tc.tile_pool(name="qk", bufs=2) as qk,
            tc.tile_pool(name="vp", bufs=2) as vp,
            tc.tile_pool(name="pp", bufs=2) as pp,
            tc.tile_pool(name="op", bufs=2) as op,
            tc.tile_pool(name="psS", bufs=3, space="PSUM") as psS,
            tc.tile_pool(name="psO", bufs=2, space="PSUM") as psO,
        ):
            pairs = {}
            heads = {}

            def emit_pair_dma(pr, startup=False):
                sb_w2 = wp.tile([128, 256], bf16, tag="w2", name=f"sb_w2_{pr}")
                sb_b2 = wp.tile([128, 2], f32, tag="b2", name=f"sb_b2_{pr}")
                sb_x2 = xi.tile([128, S], bf16, tag="x2", name=f"sb_x2_{pr}")
                # split for earlier critical-path availability of the first
                # matmuls; at startup spread across SP + ACT DGE queues
                e2 = nc.scalar if startup else nc.sync
                nc.sync.dma_start(out=sb_x2[:, 0:512], in_=xt2[pr, :, 0:512])
                e2.dma_start(out=sb_w2, in_=wqk2[pr])
                e2.dma_start(out=sb_x2[:, 512:QC], in_=xt2[pr, :, 512:QC])
                nc.sync.dma_start(out=sb_b2, in_=bqk2[pr])
                nc.sync.dma_start(out=sb_x2[:, QC:S], in_=xt2[pr, :, QC:S])
                pairs[pr] = {"w2": sb_w2, "b2": sb_b2, "x2": sb_x2}

            def emit_head_dma(h, startup=False):
                sb_xb = xb.tile([65, S], bf16, tag="xb", name=f"sb_xb_{h}", bufs=3)
                sb_wv = wp.tile([65, HD], bf16, tag="wv", name=f"sb_wv_{h}", bufs=3)
                e = nc.scalar if startup else nc.sync
                e.dma_start(out=sb_xb, in_=xtb[h])
                e.dma_start(out=sb_wv, in_=wv[h])
                heads[h] = {"xb": sb_xb, "wv": sb_wv}

            def emit_proj_qk_c(pr, c):
                pd = pairs[pr]
                if "qT" not in pd:
                    pd["qT"] = qk.tile([128, S], bf16, tag="qT", name=f"sb_qT_{pr}")
                    pd["kT"] = qk.tile([128, S], bf16, tag="kT", name=f"sb_kT_{pr}")
                x2_r = pd["x2"]
                w2_r = pd["w2"]
                # K first (scores need every k-tile; q only needs chunk 0)
                for which, col0, dst, bcol in (
                    ("k", 128, pd["kT"], 1),
                    ("q", 0, pd["qT"], 0),
                ):
                    ps = psS.tile(
                        [128, QC], f32, tag="ps", name=f"ps_{which}_{pr}_{c}"
                    )
                    for half in range(QC // 512):
                        lo = half * 512
                        nc.tensor.matmul(
                            ps[:, lo : lo + 512],
                            w2_r[:, col0 : col0 + 128],
                            x2_r[:, c * QC + lo : c * QC + lo + 512],
                            start=True,
                            stop=True,
                        )
                    nc.vector.tensor_scalar(
                        dst[:, c * QC : (c + 1) * QC],
                        ps,
                        pd["b2"][:, bcol : bcol + 1],
                        None,
                        op0=mybir.AluOpType.add,
                    )

            def emit_proj_qk(pr):
                for c in range(NQC):
                    emit_proj_qk_c(pr, c)

            def emit_proj_v(h):
                hd_ = heads[h]
                psv = psS.tile([128, QC], f32, tag="ps", name=f"psv_{h}")
                for t in range(NKT):
                    nc.tensor.matmul(
                        psv[:, t * 64 : (t + 1) * 64],
                        hd_["xb"][:, t * 128 : (t + 1) * 128],
                        hd_["wv"],
                        start=True,
                        stop=True,
                    )
                sb_vp = vp.tile([128, NKT * 65], bf16, tag="vp", name=f"sb_vp_{h}")
                vp_r = sb_vp.rearrange("p (t c) -> p t c", c=65)
                nc.gpsimd.memset(vp_r[:, :, 64:65], 1.0)
                nc.scalar.copy(
                    vp_r[:, :, 0:64], psv.rearrange("p (t c) -> p t c", c=64)
                )
                hd_["vp"] = sb_vp

            chunk = {}

            def emit_scores_tile(g, h, c, kt, eng_pat):
                pd = pairs[h // 2]
                hh = h % 2
                p0 = 64 * hh
                qT_r = pd["qT"]
                kT_r = pd["kT"]
                ps = psS.tile([128, QC], f32, tag="ps", name=f"ps_s_{g}_{kt}")
                for half in range(QC // 512):
                    lo = half * 512
                    nc.tensor.matmul(
                        ps[:, lo : lo + 512],
                        kT_r[p0 : p0 + 64, kt * 128 : (kt + 1) * 128],
                        qT_r[p0 : p0 + 64, c * QC + lo : c * QC + lo + 512],
                        start=True,
                        stop=True,
                    )
                sb_p = pp.tile([128, QC], bf16, tag=f"p{kt}", name=f"sb_p_{g}_{kt}")
                eng = eng_pat[kt]
                if eng == "A":
                    nc.scalar.activation(sb_p, ps, AF.Exp, scale=0.125)
                elif eng == "S":
                    # split one tile across both engines to fine-balance load
                    import kernel_v2 as _k
                    sp = getattr(_k, "SPLIT_COL", 550)
                    nc.scalar.activation(
                        sb_p[:, 0:sp], ps[:, 0:sp], AF.Exp, scale=0.125
                    )
                    nc.vector.tensor_scalar(
                        sb_p.bitcast(i16)[:, sp:QC],
                        ps[:, sp:QC],
                        SCH_A,
                        SCH_B,
                        ALU.mult,
                        ALU.add,
                    )
                else:
                    p_i16 = sb_p.bitcast(i16)
                    e = nc.vector if eng == "D" else nc.gpsimd
                    e.tensor_scalar(p_i16, ps, SCH_A, SCH_B, ALU.mult, ALU.add)
                chunk[g]["p"][kt] = sb_p

            def emit_chunk_start(g, h, c):
                po0 = psO.tile([128, 4 * 65], f32, tag="po", name=f"po0_{g}")
                po1 = psO.tile([128, 4 * 65], f32, tag="po", name=f"po1_{g}")
                sb_out = op.tile(
                    [128, NQB * 65], f32, tag="out", name=f"sb_out_{g}"
                )
                sb_r = op.tile([128, NQB], f32, tag="r", name=f"sb_r_{g}")
                chunk[g] = {
                    "h": h, "c": c, "p": {}, "po": (po0, po1),
                    "out": sb_out, "r": sb_r,
                }

            def emit_pv_qb(g, qb):
                st = chunk[g]
                sb_vp = heads[st["h"]]["vp"]
                po = st["po"][qb // 4]
                base = (qb % 4) * 65
                for kt in range(NKT):
                    nc.tensor.matmul(
                        po[:, base : base + 65],
                        st["p"][kt][:, qb * 128 : (qb + 1) * 128],
                        sb_vp[:, kt * 65 : (kt + 1) * 65],
                        start=(kt == 0),
                        stop=(kt == NKT - 1),
                    )
                if qb % 4 != 3:
                    return
                # normalize a 4-qb group: one batched reciprocal of the four
                # denominator columns, then one broadcast tensor_tensor
                # multiply over the whole [128, 4*65] po tile (den*r = 1 in
                # the spare columns; the out DMA skips them)
                half = qb // 4
                nc.vector.reciprocal(
                    st["r"][:, half * 4 : half * 4 + 4], po[:, 64::65]
                )
                r_b = (
                    st["r"][:, half * 4 : half * 4 + 4]
                    .unsqueeze(-1)
                    .broadcast_to([128, 4, 65])
                )
                nc.vector.tensor_tensor(
                    st["out"][:, half * 260 : (half + 1) * 260].rearrange(
                        "p (qb c) -> p qb c", c=65
                    ),
                    po.rearrange("p (qb c) -> p qb c", c=65),
                    r_b,
                    op=mybir.AluOpType.mult,
                )

            def emit_out_dma(g, split=0):
                st = chunk[g]
                h, c = st["h"], st["c"]
                nhalf = split if split else 1
                oh_all = st["out"].rearrange("p (qb c) -> p qb c", c=65)
                for i in range(nhalf):
                    qb0 = i * (NQB // nhalf)
                    qb1 = (i + 1) * (NQB // nhalf)
                    out_r = out[
                        h, c * QC + qb0 * 128 : c * QC + qb1 * 128, :
                    ].rearrange("(qb p) d -> p qb d", p=128)
                    oh_r = oh_all[:, qb0:qb1, 0:64]
                    nc.sync.dma_start(out=out_r, in_=oh_r)
                del st["p"]

            emit_pair_dma(0, startup=True)
            emit_head_dma(0, startup=True)
            emit_head_dma(1, startup=True)
            # tiny dummy exp issued after the startup DMA configs: pulls the
            # one-time ACT table load (~1.3us) off chunk 0's critical path
            warm = wp.tile([1, 2], f32, tag="warm", name="warm")
            nc.gpsimd.memset(warm, 0.0)
            nc.scalar.activation(warm, warm, AF.Exp, scale=1.0)
            emit_proj_qk(0)
            emit_proj_v(0)
            gs = [(h, c) for h in range(HEADS_PER_CORE) for c in range(NQC)]
            prev = None
            for g, (h, c) in enumerate(gs):
                if g == len(gs) - 1:
                    pat = EXP_ENG_LAST
                elif g == 0:
                    pat = EXP_ENG_FIRST
                else:
                    pat = EXP_ENG
                emit_chunk_start(g, h, c)
                # spread the previous chunk's 8 PV q-blocks evenly across all
                # 16 score emissions so PE has fill-in work wherever the
                # psum-slot rotation would otherwise stall on a late exp
                pv_sched = {3: 0, 5: 1, 7: 2, 9: 3, 10: 4, 12: 5, 13: 6, 15: 7}
                for kt in range(NKT):
                    emit_scores_tile(g, h, c, kt, pat)
                    if prev is not None and kt in pv_sched:
                        emit_pv_qb(prev, pv_sched[kt])
                if prev is not None:
                    emit_out_dma(prev)
                if c == 0:
                    if h % 2 == 0:  # after first chunk of a pair's first head
                        if h + 2 < HEADS_PER_CORE:
                            emit_pair_dma(h // 2 + 1)
                            emit_head_dma(h + 2)
                            emit_head_dma(h + 3)
                    else:  # first chunk of a pair's second head: next pair proj
                        if h + 1 < HEADS_PER_CORE:
                            emit_proj_qk(h // 2 + 1)
                            emit_proj_v(h + 1)
                elif c == 1 and h % 2 == 0:
                    emit_proj_v(h + 1)
                prev = g
            for qb in range(NQB):
                emit_pv_qb(prev, qb)
            emit_out_dma(prev, split=4)

    return nc


def _pack_core_inputs(core, x, Wq, bq, Wk, bk, Wv, bv):
    """Host-side packing of one core's DRAM parameters."""
    import ml_dtypes

    bf = ml_dtypes.bfloat16
    xh = x.reshape(B, H, S, HD)
    ones = np.ones((1, S), np.float32)
    xt2 = np.zeros((N_PAIRS, 128, S), np.float32)
    wqk2 = np.zeros((N_PAIRS, 128, 256), np.float32)
    bqk2 = np.zeros((N_PAIRS, 128, 2), np.float32)
    xtb = np.empty((HEADS_PER_CORE, 65, S), np.float32)
    wv_p = np.empty((HEADS_PER_CORE, 65, HD), np.float32)
    for slot in range(HEADS_PER_CORE):
        flat = core * HEADS_PER_CORE + slot
        b, h = divmod(flat, H)
        pr, hh = divmod(slot, 2)
        xt2[pr, 64 * hh : 64 * hh + 64] = xh[b, h].T
        wqk2[pr, 64 * hh : 64 * hh + 64, 64 * hh : 64 * hh + 64] = Wq[h].T
        wqk2[pr, 64 * hh : 64 * hh + 64, 128 + 64 * hh : 128 + 64 * hh + 64] = Wk[h].T
        bqk2[pr, 64 * hh : 64 * hh + 64, 0] = bq[h]
        bqk2[pr, 64 * hh : 64 * hh + 64, 1] = bk[h]
        xtb[slot] = np.concatenate([xh[b, h].T, ones], axis=0)
        wv_p[slot] = np.concatenate([Wv[h].T, bv[h][None, :]], axis=0)
    return {
        "xt2": np.ascontiguousarray(xt2).astype(bf),
        "wqk2": np.ascontiguousarray(wqk2).astype(bf),
        "bqk2": np.ascontiguousarray(bqk2),
        "xtb": np.ascontiguousarray(xtb).astype(bf),
        "wv": np.ascontiguousarray(wv_p).astype(bf),
    }


def kernel(x, Wq, bq, Wk, bk, Wv, bv):
    global LAST_RESULTS
    import os

    from concourse.bass_utils import run_bass_kernel_spmd

    x = np.asarray(x, dtype=np.float32)
    Wq = np.asarray(Wq, dtype=np.float32)
    bq = np.asarray(bq, dtype=np.float32)
    Wk = np.asarray(Wk, dtype=np.float32)
    bk = np.asarray(bk, dtype=np.float32)
    Wv = np.asarray(Wv, dtype=np.float32)
    bv = np.asarray(bv, dtype=np.float32)

    in_maps = [
        _pack_core_inputs(core, x, Wq, bq, Wk, bk, Wv, bv) for core in range(N_CORES)
    ]

    nc = _build_bass()
    nc.finalize()
    trace = bool(os.environ.get("KERNEL_TRACE"))
    LAST_RESULTS = run_bass_kernel_spmd(
        nc, in_maps, core_ids=list(range(N_CORES)), trace=trace
    )

    final = np.empty((B, S, D), dtype=np.float32)
    for core in range(N_CORES):
        res = LAST_RESULTS.results[core]["out"]
        for slot in range(HEADS_PER_CORE):
            flat = core * HEADS_PER_CORE + slot
            b, h = divmod(flat, H)
            final[b, :, h * HD : (h + 1) * HD] = res[slot]
    return final
